# revision 1
# baseline (speedup 1.0000x reference)
"""Trainium2 Bass kernel for LeViT-style attention (nn_Attention_27805618275053).

reference math:
    qkv  = x @ w_qkv.T + b_qkv                  # [B,N,2*H*KD+H*VD]
    q,k,v split per head; s = q k^T * SCALE + bias[h, q, k]
    p = softmax(s, axis=keys);  o = p v;  out = o @ w_proj.T + b_proj

Strategy: pure data-parallel over batch (B=256 -> 32 per core, 8 cores, no
collectives).  All matmuls in bf16 (fp32 PSUM accumulation), validated host-side
at rel_err ~6e-3.

Host-side exact algebraic folds:
  - SCALE folded into w_q, b_q.
  - k-bias dropped (softmax invariant under per-query shift).
  - v-bias folded into b_proj (softmax weights sum to 1):
        b_proj_eff = b_proj + b_v @ w_proj.T
  - attention bias applied as e = exp(s) * exp(bias)^T  with exp(bias^T)
    precomputed host-side (replicated, tiny).

Per-core layout (per batch of a G=8 group):
  xT [128x3, 1568]      group x^T via DMA-transpose; next group's xT is
      prefetched mid-group on the ACT HWDGE ring (qActDynamicHW) so the
      1.2MB of transpose traffic never queues ahead of the
      latency-critical rd round-trips on SP's ring (10us stall otherwise)
  qk GEMM per 2-batch pair: psum [128 feats, 392]; q copies (ACT, +bias)
      and k copies (DVE) keep full 128-partition 4-head tiles; k tiles
      [128, 512] have zero-padded 256-col per-batch blocks (zero keys ->
      eb=0, so kc1 matmuls can use full K=128)
  v  [196(2 chunks), 1024] = xT chunks (stationary) @ wv^T (moving)
  per head h: sT[keys, 196q] = kT_h (stationary [32,128] at PE row-tile
      position 32*(h%4)) @ qT_h (moving), both key chunks into one
      [128, 392] psum; one ACT exp; one DVE mult by exp(bias^T);
      sel_h-matmuls accumulate all heads' key-sums into one [8,196] psum;
      o^T matmuls run immediately (two heads share a [128,392] psum bank
      so ps_o_bufs*2 heads stay in flight)
  per batch: one DVE reciprocal_approx_fast [8,196] (~18 bits, 5x faster
      than reciprocal); rd broadcast to [128, 8, 196] via DRAM round-trip
      DMA (engines cannot read non-zero partition bases); per-pair DVE
      normalize into oT_all [128, 8 heads, 1568] bf16
  GEMM2 over group-flattened tokens: out[tok, 384] = oT_all slices
      (stationary) @ wp^T_h (moving), accumulated over 8 heads; DVE adds
      b_proj; bf16 stores ride GpSimd's SWDGE queue to keep SP's queue
      free (host converts back to f32).

HW exec: 489us baseline -> ~452-457us (measurements drift ~15% with the
device's thermal/P0 state; compare configs back-to-back only).
"""

import json
import math
import os
from contextlib import ExitStack

import ml_dtypes
import numpy as np

import concourse.bass as bass
import concourse.tile as tile
from concourse import bacc, mybir
from concourse.bass_utils import run_bass_kernel_spmd

B, N, C = 256, 196, 384
H, KD, VD = 8, 32, 128
SCALE = KD ** -0.5
NCORES = 8
BL = B // NCORES          # batches per core
G = 8                     # batches per group (GEMM2 token-flattening)
NG = BL // G              # groups per core
NTOK_G = G * N            # 1568 flat tokens per group
KC = [(0, 128), (128, 68)]  # key/token chunks of N=196

F32 = mybir.dt.float32
BF16 = mybir.dt.bfloat16
BF16_NP = ml_dtypes.bfloat16


def _ceil_div(a, b):
    return (a + b - 1) // b


DEFAULT_CFG = dict(
    xt_bufs=2, qkt_bufs=8, v_bufs=4, et_bufs=4, eb_bufs=10, rd_bufs=4,
    ot_bufs=2, out_bufs=3, ps_mm_bufs=2, ps_s_bufs=2, ps_d_bufs=1, ps_o_bufs=3,
    ps_g2_bufs=0,
    eb_gpsimd=0, selcol=0, srow=1, g2_interleave=0, xt_prefetch=1, g2_flip=0,
    xt_act=1, qcopy_dve=0, kcopy_act=0, xt_pre_bi=3, bp_host=1,
)


def _merged_cfg():
    return {**DEFAULT_CFG, **json.loads(os.environ.get("KCFG", "{}"))}


def build_graph(cfg=None):
    cfg = {**DEFAULT_CFG, **(cfg or {})}
    nc = bacc.Bacc("TRN2", target_bir_lowering=False, debug=False)

    # ---- DRAM parameters (per-core shard) ----
    x_d = nc.dram_tensor("x", [BL * N, C], BF16, kind="ExternalInput").ap()
    wqk_d = nc.dram_tensor("wqk_t", [C, 2 * H * KD], BF16, kind="ExternalInput").ap()
    wv_d = nc.dram_tensor("wv_t", [C, H * VD], BF16, kind="ExternalInput").ap()
    wp_d = nc.dram_tensor("wp_t", [H * VD, C], BF16, kind="ExternalInput").ap()
    bq_d = nc.dram_tensor("bq", [H * KD], F32, kind="ExternalInput").ap()
    bp_d = nc.dram_tensor("bp", [C], F32, kind="ExternalInput").ap()
    # exp(bias)^T packed per head: [:, 0:196] = keys 0:128 (rows 0:128),
    # [:, 196:392] = keys 128:196 (rows 0:68), zeros elsewhere.
    expb_d = nc.dram_tensor("expb_p", [H, 128, 2 * N], BF16, kind="ExternalInput").ap()
    if cfg["g2_flip"]:
        # transposed output [C, tokens]; host transposes back
        out_d = nc.dram_tensor("out", [C, BL * N], BF16,
                               kind="ExternalOutput").ap()
    else:
        out_d = nc.dram_tensor("out", [BL * N, C], BF16,
                               kind="ExternalOutput").ap()

    with tile.TileContext(nc) as tc, ExitStack() as ctx:
        singles = ctx.enter_context(tc.tile_pool(name="singles", bufs=1))
        xt_pool = ctx.enter_context(tc.tile_pool(name="xt", bufs=cfg["xt_bufs"]))
        qkt_pool = ctx.enter_context(tc.tile_pool(name="qkt", bufs=cfg["qkt_bufs"]))
        v_pool = ctx.enter_context(tc.tile_pool(name="v", bufs=cfg["v_bufs"]))
        et_pool = ctx.enter_context(tc.tile_pool(name="et", bufs=cfg["et_bufs"]))
        eb_pool = ctx.enter_context(tc.tile_pool(name="eb", bufs=cfg["eb_bufs"]))
        rd_pool = ctx.enter_context(tc.tile_pool(name="rd", bufs=cfg["rd_bufs"]))
        ot_pool = ctx.enter_context(tc.tile_pool(name="ot", bufs=cfg["ot_bufs"]))
        out_pool = ctx.enter_context(tc.tile_pool(name="outp", bufs=cfg["out_bufs"]))

        rdd_pool = ctx.enter_context(tc.tile_pool(name="rdd", bufs=8, space="DRAM"))
        ps_mm = ctx.enter_context(
            tc.tile_pool(name="ps_mm", bufs=cfg["ps_mm_bufs"], space="PSUM"))
        ps_s = ctx.enter_context(
            tc.tile_pool(name="ps_s", bufs=cfg["ps_s_bufs"], space="PSUM"))
        ps_d = ctx.enter_context(
            tc.tile_pool(name="ps_d", bufs=cfg["ps_d_bufs"], space="PSUM"))
        ps_o = ctx.enter_context(
            tc.tile_pool(name="ps_o", bufs=cfg["ps_o_bufs"], space="PSUM"))

        def load_xt(g, eng=None):
            """x^T tiles [128, 1568] for group g via DMA transpose.

            eng selects the HWDGE ring: prefetches ride ACT's ring
            (qActDynamicHW) so the 1.2MB of transpose traffic never queues
            ahead of the latency-critical rd round-trips on SP's ring.
            """
            eng = eng if eng is not None else nc.sync
            tiles = []
            for cc in range(3):
                t = xt_pool.tile([128, NTOK_G], BF16, tag=f"xt{cc}",
                                 name=f"xt{cc}_{g}")
                eng.dma_start_transpose(
                    out=t[:],
                    in_=x_d[g * NTOK_G:(g + 1) * NTOK_G,
                            cc * 128:(cc + 1) * 128],
                )
                tiles.append(t)
            return tiles

        # ---- resident constants ----
        # Warm the ACT Exp table first: the lazy table load (~1.3us + DMA)
        # otherwise lands right before the first real exp and stalls the
        # whole attention pipeline ~16us into the run.
        warm = singles.tile([1, 1], F32, tag="warm")
        nc.vector.memset(warm[:], 0.0)
        nc.scalar.activation(warm[:], warm[:],
                             mybir.ActivationFunctionType.Exp)
        wqk_s = []  # 3 tiles [128, 512] (q cols 0:256 | k cols 256:512)
        for cc in range(3):
            t = singles.tile([128, 2 * H * KD], BF16, tag=f"wqk{cc}")
            nc.sync.dma_start(out=t[:], in_=wqk_d[cc * 128:(cc + 1) * 128, :])
            wqk_s.append(t)
        xt_cache = {0: load_xt(0)}
        # remaining constants ordered by first use: wv (v GEMM ~20us), bq
        # (first qk copy), expb (first eb mult), bp, then wp (first GEMM2,
        # ~100us in) — the SP DMA queue drains in emission order.
        wv_s = []   # 3 tiles [128, 1024]
        for cc in range(3):
            tv = singles.tile([128, H * VD], BF16, tag=f"wv{cc}")
            nc.sync.dma_start(out=tv[:], in_=wv_d[cc * 128:(cc + 1) * 128, :])
            wv_s.append(tv)
        # q bias (scaled) as per-partition columns: 2 tiles [128, 1]
        bq_s = []
        for fc in range(2):
            t = singles.tile([128, 1], F32, tag=f"bq{fc}")
            nc.sync.dma_start(
                out=t[:],
                in_=bq_d[fc * 128:(fc + 1) * 128].rearrange("(p o) -> p o", o=1),
            )
            bq_s.append(t)
        # exp(bias)^T packed [128, 392] per head (both key chunks side by side)
        expb_s = singles.tile([128, H, 2 * N], BF16, tag="expb")
        for h in range(H):
            nc.sync.dma_start(out=expb_s[:, h, :], in_=expb_d[h, :, :])
        wp_s = []   # 8 tiles [128, 384] (first needed at GEMM2, load last)
        for h in range(H):
            t = singles.tile([128, C], BF16, tag=f"wp{h}")
            nc.sync.dma_start(out=t[:], in_=wp_d[h * 128:(h + 1) * 128, :])
            wp_s.append(t)
        if cfg["g2_flip"]:
            # proj bias as per-partition columns [128, 3]: bpc[p, cc] =
            # bp[cc*128 + p], for the transposed-GEMM2 ACT/DVE bias add
            bp_s = singles.tile([128, 3], F32, tag="bp")
            nc.sync.dma_start(
                out=bp_s[:],
                in_=bp_d.rearrange("(cc p) -> p cc", p=128),
            )
        else:
            # proj bias broadcast to all partitions [128, 384]
            bp_s = singles.tile([128, C], F32, tag="bp")
            nc.sync.dma_start(
                out=bp_s[:],
                in_=bass.AP(tensor=bp_d.tensor, offset=bp_d.offset,
                            ap=[[0, 128]] + bp_d.ap),
            )
        # indicator stationaries for per-head denominator rows.
        # selcol=1: head h goes to PE col-tile h//2 (tile_position=
        # (0, 32*(h//2))), row h%2 within it, so pairs (2p, 2p+1) land on
        # adjacent PSUM partitions 32p, 32p+1.  selcol=0: legacy [128, 8]
        # indicators accumulating all heads into rows 0..7 of one bank.
        sel_w = 2 if cfg["selcol"] else H
        sel_s = []
        for h in range(H):
            t = singles.tile([128, sel_w], BF16, tag=f"sel{h}")
            nc.vector.memset(t[:], 0.0)
            col = h % 2 if cfg["selcol"] else h
            nc.vector.memset(t[:, col:col + 1], 1.0)
            sel_s.append(t)

        n_tc = _ceil_div(NTOK_G, 128)  # 13 token chunks per group for GEMM2

        for g in range(NG):
            tok0 = g * NTOK_G
            # ---- x^T for the whole group: 3 tiles [128, 1568] ----
            xT = xt_cache.pop(g) if g in xt_cache else load_xt(g)

            # normalized oT for the group: [128 vd, 8 heads, 1568 q]
            # (legacy layout; with g2_flip oT lives in per-pair tiles instead)
            if not cfg["g2_flip"]:
                oT_all = ot_pool.tile([128, H, NTOK_G], BF16, tag="ot",
                                      name="oT_all")

            # GEMM2 chunk c (tokens [128c, 128c+tn)) only needs batches
            # 0..ceil(128(c+1)/196)-1 normalized; emit each chunk right after
            # the batch that completes it so GEMM2 pipelines across the group
            # instead of piling up (PE-idle + HAM re-throttle) at group end.
            def emit_g2(chunks):
                for tci in chunks:
                    t0 = tci * 128
                    tn = min(128, NTOK_G - t0)
                    if cfg["ps_g2_bufs"]:
                        ps = ps_mm.tile([128, C], F32, tag="g2",
                                        bufs=cfg["ps_g2_bufs"], name="psg2")
                    else:
                        ps = ps_mm.tile([128, C], F32, tag="mm", name="psg2")
                    for h in range(H):
                        nc.tensor.matmul(
                            ps[:tn, :],
                            lhsT=oT_all[:, h, t0:t0 + tn],
                            rhs=wp_s[h][:],
                            start=(h == 0), stop=(h == H - 1),
                        )
                    ot = out_pool.tile([128, C], BF16, tag="out")
                    if cfg["bp_host"]:
                        # b_proj added host-side; plain copy runs ~2x faster
                        # on DVE than the tensor_tensor add
                        nc.vector.tensor_copy(ot[:tn, :], ps[:tn, :])
                    else:
                        nc.vector.tensor_add(ot[:tn, :], ps[:tn, :],
                                             bp_s[:tn, :])
                    # stores ride GpSimd's SWDGE queue to keep SP's queue free
                    nc.gpsimd.dma_start(
                        out=out_d[tok0 + t0:tok0 + t0 + tn, :], in_=ot[:tn, :]
                    )

            g2_done = 0
            qkT = None
            for bi in range(G):
                bt0 = bi * N  # batch token offset within group
                bq0 = (bi % 2) * N  # this batch's column offset in the pair

                if bi % 2 == 0:
                    if cfg["g2_flip"]:
                        # normalized oT for this 2-batch pair
                        ot_pair = ot_pool.tile([128, H, 2 * N], BF16,
                                               tag="ot", name="ot_pair")
                    # ---- GEMM1 qk^T for 2 batches: psum [128 feats, 392] ----
                    # 4 tiles [128, .]: 0..1 = q (heads 0-3, 4-7), 2..3 = k
                    qkT = []
                    for fc in range(4):
                        ps = ps_mm.tile([128, 2 * N], F32, tag="mm")
                        for cc in range(3):
                            nc.tensor.matmul(
                                ps[:],
                                lhsT=wqk_s[cc][:, fc * 128:(fc + 1) * 128],
                                rhs=xT[cc][:, bt0:bt0 + 2 * N],
                                start=(cc == 0), stop=(cc == 2),
                            )
                        if fc < 2:
                            # q: [128, 392], add (scaled) bias in the copy
                            t = qkt_pool.tile([128, 2 * N], BF16, tag="qkt")
                            if cfg["qcopy_dve"]:
                                nc.vector.tensor_scalar_add(
                                    t[:], ps[:], bq_s[fc][:])
                            else:
                                nc.scalar.activation(
                                    t[:], ps[:],
                                    mybir.ActivationFunctionType.Identity,
                                    bias=bq_s[fc][:], scale=1.0,
                                )
                        else:
                            # k: [128, 512] = two 256-col per-batch blocks,
                            # keys padded with zero columns so kc1 s-matmuls
                            # can use full K=128 (zero keys -> s=0, exp->1,
                            # *expb(0 pad)=0: eb fully defined).
                            t = qkt_pool.tile([128, 512], BF16, tag="qktk")
                            tv = t[:].rearrange("p (b n) -> p b n", b=2)
                            if cfg["kcopy_act"]:
                                nc.scalar.copy(
                                    tv[:, :, 0:N],
                                    ps[:].rearrange("p (b n) -> p b n", b=2),
                                )
                            else:
                                nc.vector.tensor_copy(
                                    tv[:, :, 0:N],
                                    ps[:].rearrange("p (b n) -> p b n", b=2),
                                )
                            nc.vector.memset(tv[:, :, N:256], 0.0)
                        qkT.append(t)

                # ---- GEMM1 v: [196(2 chunks), 1024] ----
                v_s = []
                for ci, (t0, tn) in enumerate(KC):
                    vt = v_pool.tile([tn, H * VD], BF16, tag="v",
                                     name=f"v{ci}")
                    for nn in range(2):
                        ps = ps_mm.tile([128, 512], F32, tag="mm")
                        for cc in range(3):
                            nc.tensor.matmul(
                                ps[:tn, :],
                                lhsT=xT[cc][:, bt0 + t0:bt0 + t0 + tn],
                                rhs=wv_s[cc][:, nn * 512:(nn + 1) * 512],
                                start=(cc == 0), stop=(cc == 2),
                            )
                        if nn == 0:  # split the two copies across ACT and DVE
                            nc.scalar.copy(
                                vt[:, nn * 512:(nn + 1) * 512], ps[:tn, :]
                            )
                        else:
                            nc.vector.tensor_copy(
                                vt[:, nn * 512:(nn + 1) * 512], ps[:tn, :]
                            )
                    v_s.append(vt)

                # denominator banks: with selcol, pair p's key-sums land on
                # PSUM rows 0:2, free slice p%2, of bank p//2 ([2, 2, 196]
                # tiles, two pair-groups per bank).  Each pair is its own
                # accumulation group so rd_p is ready as soon as head 2p+1's
                # eb is done (no batch-end pile-up, o banks free early).
                if cfg["selcol"]:
                    ps_dd = [ps_d.tile([2, 2, N], F32, tag="d", name=f"psd{i}")
                             for i in range(2)]
                else:
                    ps_dd = ps_d.tile([H, N], F32, tag="d")
                ps_op = {}
                for h in range(H):
                    hg, hi = divmod(h, 4)
                    qT = qkT[hg]
                    kT = qkT[2 + hg]
                    hsl = slice(hi * 32, (hi + 1) * 32)
                    ps_st = ps_s.tile([128, 2 * N], F32, tag="s", name=f"pss{h}")
                    kb0 = (bi % 2) * 256  # this batch's block in the k tile
                    for ci in range(2):
                        # K=32 row tile at position 32*hi: up to 4 heads run
                        # concurrently in distinct PE row groups
                        nc.tensor.matmul(
                            ps_st[:, ci * N:(ci + 1) * N],
                            lhsT=kT[hsl, kb0 + ci * 128:kb0 + (ci + 1) * 128],
                            rhs=qT[hsl, bq0:bq0 + N],
                            start=True, stop=True,
                            **({"tile_position": (hi * 32, 0)}
                               if cfg["srow"] else {}),
                        )
                    et = et_pool.tile([128, 2 * N], BF16, tag="e", name=f"et{h}")
                    nc.scalar.activation(
                        et[:], ps_st[:], mybir.ActivationFunctionType.Exp
                    )
                    eb = eb_pool.tile([128, 2 * N], BF16, tag="eb", name=f"eb{h}")
                    if cfg["eb_gpsimd"]:
                        nc.gpsimd.tensor_mul(eb[:], et[:], expb_s[:, h, :])
                    else:
                        nc.vector.tensor_mul(eb[:], et[:], expb_s[:, h, :])
                    if cfg["selcol"]:
                        p = h // 2
                        psd = ps_dd[p // 2][:, p % 2, :]  # [2, 196] region
                        for ci in range(2):
                            nc.tensor.matmul(
                                psd,
                                lhsT=sel_s[h][:],
                                rhs=eb[:, ci * N:(ci + 1) * N],
                                start=(h % 2 == 0 and ci == 0),
                                stop=(h % 2 == 1 and ci == 1),
                            )
                    else:
                        for ci in range(2):
                            nc.tensor.matmul(
                                ps_dd[:],
                                lhsT=sel_s[h][:],
                                rhs=eb[:, ci * N:(ci + 1) * N],
                                start=(h == 0 and ci == 0),
                                stop=(h == H - 1 and ci == 1),
                            )
                    # o^T immediately (doesn't need rd); two heads share one
                    # [128, 392] PSUM bank so 2*ps_o_bufs heads stay in flight
                    if h % 2 == 0:
                        ps_op[h // 2] = ps_o.tile(
                            [128, 2 * N], F32, tag="o", name=f"pso{h // 2}"
                        )
                    pso = ps_op[h // 2]
                    for ci, (k0, kn) in enumerate(KC):
                        nc.tensor.matmul(
                            pso[:, (h % 2) * N:(h % 2 + 1) * N],
                            lhsT=v_s[ci][:, h * VD:(h + 1) * VD],
                            rhs=eb[:kn, ci * N:(ci + 1) * N],
                            start=(ci == 0), stop=(ci == 1),
                        )
                    if cfg["selcol"] and h % 2 == 1:
                        # pair complete: reciprocal + broadcast + normalize now
                        p = h // 2
                        rd2 = rd_pool.tile([2, N], F32, tag="rd", name="rd2")
                        # ~18-bit reciprocal, 5x faster than vector.reciprocal;
                        # denominators are sums of exps (no edge cases)
                        nc.vector.reciprocal_approx_fast(
                            out=rd2[:], in_=ps_dd[p // 2][:, p % 2, :])
                        rdd = rdd_pool.tile([2, N], F32, tag="rdd")
                        nc.sync.dma_start(out=rdd[:], in_=rd2[:])
                        rdb = rd_pool.tile([128, 2, N], F32, tag="rdb",
                                           name="rdb2")
                        rdd_ap = rdd[:]
                        nc.sync.dma_start(
                            out=rdb[:],
                            in_=bass.AP(tensor=rdd_ap.tensor,
                                        offset=rdd_ap.offset,
                                        ap=[[0, 128]] + list(rdd_ap.ap)),
                        )
                        # normalize both heads of the pair in one op
                        o_dst = (ot_pair[:, 2 * p:2 * p + 2, bq0:bq0 + N]
                                 if cfg["g2_flip"] else
                                 oT_all[:, 2 * p:2 * p + 2, bt0:bt0 + N])
                        nc.vector.tensor_mul(
                            o_dst,
                            ps_op[p][:].rearrange("v (b n) -> v b n", b=2),
                            rdb[:],
                        )

                if not cfg["selcol"]:
                    rd8 = rd_pool.tile([H, N], F32, tag="rd", name="rd8")
                    nc.vector.reciprocal_approx_fast(
                        out=rd8[:], in_=ps_dd[:])
                    rdd8 = rdd_pool.tile([H, N], F32, tag="rdd8")
                    nc.sync.dma_start(out=rdd8[:], in_=rd8[:])
                    rdb_all = rd_pool.tile([128, H, N], F32, tag="rdb",
                                           name="rdb_all")
                    rdd_ap = rdd8[:]
                    nc.sync.dma_start(
                        out=rdb_all[:],
                        in_=bass.AP(tensor=rdd_ap.tensor, offset=rdd_ap.offset,
                                    ap=[[0, 128]] + list(rdd_ap.ap)),
                    )
                    for p in range(H // 2):
                        o_dst = (ot_pair[:, 2 * p:2 * p + 2, bq0:bq0 + N]
                                 if cfg["g2_flip"] else
                                 oT_all[:, 2 * p:2 * p + 2, bt0:bt0 + N])
                        nc.vector.tensor_mul(
                            o_dst,
                            ps_op[p][:].rearrange("v (b n) -> v b n", b=2),
                            rdb_all[:, 2 * p:2 * p + 2, :],
                        )

                # prefetch next group's x^T mid-group so its DMA-transpose
                # latency is hidden under this group's compute
                if cfg["xt_prefetch"] and bi == cfg["xt_pre_bi"] and g + 1 < NG:
                    xt_cache[g + 1] = load_xt(
                        g + 1, eng=nc.scalar if cfg["xt_act"] else None)

                if cfg["g2_flip"] and bi % 2 == 1:
                    # ---- transposed GEMM2 for this pair: outT[C, 392] ----
                    # stationary wp blocks [128 vd, 128 c], moving oT
                    # [128 vd, 392 tokens]; accumulate over heads.  Emitted
                    # per pair, so GEMM2 pipelines across the whole group.
                    ptok0 = tok0 + (bi - 1) * N
                    for cci in range(3):
                        ps = ps_mm.tile([128, 2 * N], F32, tag="g2",
                                        bufs=cfg["ps_g2_bufs"], name="psg2f")
                        for h in range(H):
                            nc.tensor.matmul(
                                ps[:],
                                lhsT=wp_s[h][:, cci * 128:(cci + 1) * 128],
                                rhs=ot_pair[:, h, :],
                                start=(h == 0), stop=(h == H - 1),
                            )
                        otq = out_pool.tile([128, 2 * N], BF16, tag="out")
                        nc.vector.tensor_scalar_add(
                            otq[:], ps[:], bp_s[:, cci:cci + 1])
                        # stores ride GpSimd's SWDGE queue
                        nc.gpsimd.dma_start(
                            out=out_d[cci * 128:(cci + 1) * 128,
                                      ptok0:ptok0 + 2 * N],
                            in_=otq[:],
                        )

                # GEMM2 chunks whose tokens are fully normalized by now
                if not cfg["g2_flip"] and cfg["g2_interleave"] == 1:
                    if bi < G - 1:
                        g2_ready = (N * (bi + 1)) // 128
                    else:
                        g2_ready = n_tc
                    emit_g2(range(g2_done, g2_ready))
                    g2_done = g2_ready
                elif not cfg["g2_flip"] and cfg["g2_interleave"] == 2:
                    # coarse split: half the chunks mid-group, rest at end —
                    # shrinks the group-end GEMM2 tail without per-batch WAR
                    # serialization on oT_all
                    if bi == 5:
                        emit_g2(range(0, 6))
                        g2_done = 6

            if not cfg["g2_flip"] and cfg["g2_interleave"] != 1:
                emit_g2(range(g2_done, n_tc))

    nc.compile()
    return nc


def prep_inputs(x, w_qkv, b_qkv, w_proj, b_proj, attn_biases, bias_idxs):
    """Host-side weight permutation / folding. Returns per-core in_maps."""
    x = np.asarray(x, np.float32)
    w_qkv = np.asarray(w_qkv, np.float32)
    b_qkv = np.asarray(b_qkv, np.float32)
    w_proj = np.asarray(w_proj, np.float32)
    b_proj = np.asarray(b_proj, np.float32)
    attn_biases = np.asarray(attn_biases, np.float32)
    bias_idxs = np.asarray(bias_idxs)

    w = w_qkv.reshape(H, 2 * KD + VD, C)
    b = b_qkv.reshape(H, 2 * KD + VD)
    wq = w[:, :KD].reshape(H * KD, C) * SCALE
    bq = b[:, :KD].reshape(-1) * SCALE
    wk = w[:, KD:2 * KD].reshape(H * KD, C)
    wv = w[:, 2 * KD:].reshape(H * VD, C)
    bv = b[:, 2 * KD:].reshape(-1)

    wqk_t = np.concatenate([wq, wk], axis=0).T.copy()          # [384, 512]
    wv_t = wv.T.copy()                                         # [384, 1024]
    wp_t = w_proj.T.copy()                                     # [1024, 384]
    bp_eff = b_proj + bv @ w_proj.T                            # [384]
    bias = attn_biases[:, bias_idxs]                           # [H, q, k]
    expb_t = np.exp(bias.transpose(0, 2, 1))                   # [H, keys, q]
    # pack per head into [128, 392]: keys 0:128 in cols 0:196 and keys
    # 128:196 in cols 196:392 (rows 0:68), zeros elsewhere.
    expb_p = np.zeros((H, 128, 2 * N), np.float32)
    expb_p[:, :, :N] = expb_t[:, :128, :]
    expb_p[:, :68, N:] = expb_t[:, 128:, :]

    global _BP_EFF
    _BP_EFF = bp_eff.astype(np.float32)
    shared = {
        "wqk_t": wqk_t.astype(BF16_NP),
        "wv_t": wv_t.astype(BF16_NP),
        "wp_t": wp_t.astype(BF16_NP),
        "bq": bq.astype(np.float32),
        "bp": bp_eff.astype(np.float32),
        "expb_p": np.ascontiguousarray(expb_p).astype(BF16_NP),
    }
    in_maps = []
    for i in range(NCORES):
        xi = x[i * BL:(i + 1) * BL].reshape(BL * N, C).astype(BF16_NP)
        in_maps.append({"x": np.ascontiguousarray(xi), **shared})
    return in_maps


_CACHED_NC = None


def _get_nc():
    global _CACHED_NC
    if _CACHED_NC is None:
        _CACHED_NC = build_graph(json.loads(os.environ.get("KCFG", "{}")))
    return _CACHED_NC


_BP_EFF = None


def postprocess_outs(res):
    """Per-core DRAM 'out' -> full [B, N, C] float32."""
    cfg = _merged_cfg()
    outs = []
    for i in range(NCORES):
        o = np.asarray(res.results[i]["out"], np.float32)
        if cfg["g2_flip"]:
            o = o.T  # [C, BL*N] -> [BL*N, C]
        outs.append(np.ascontiguousarray(o).reshape(BL, N, C))
    full = np.concatenate(outs, axis=0)
    if cfg["bp_host"]:
        full += _BP_EFF  # folded proj bias (b_proj + b_v @ w_proj.T)
    return full


def kernel(x, w_qkv, b_qkv, w_proj, b_proj, attn_biases, bias_idxs, **_kw):
    in_maps = prep_inputs(x, w_qkv, b_qkv, w_proj, b_proj, attn_biases, bias_idxs)
    nc = _get_nc()
    res = run_bass_kernel_spmd(nc, in_maps, core_ids=list(range(NCORES)))
    return postprocess_outs(res)


if __name__ == "__main__":
    rng = np.random.default_rng(0)
    ins = {
        "x": rng.standard_normal((B, N, C), dtype=np.float32),
        "w_qkv": rng.standard_normal((2 * H * KD + H * VD, C), dtype=np.float32)
        / math.sqrt(C),
        "b_qkv": rng.standard_normal(2 * H * KD + H * VD).astype(np.float32) * 0.01,
        "w_proj": rng.standard_normal((C, H * VD), dtype=np.float32)
        / math.sqrt(H * VD),
        "b_proj": rng.standard_normal(C).astype(np.float32) * 0.01,
        "attn_biases": rng.standard_normal((H, 196)).astype(np.float32) * 0.02,
        "bias_idxs": rng.integers(0, 196, (N, N)).astype(np.int32),
    }
    out = kernel(**ins)
    print("out", out.shape, out.dtype)

